# revision 1
# baseline (speedup 1.0000x reference)
"""Masked mean-pool over ragged sequences on 8 Trainium2 NeuronCores.

features [32, 2048, 1024] f32, lengths [32] i32 -> pooled [32, 1024] f32
pooled[b] = mean(features[b, :lengths[b]], axis=0)

Strategy: only the first lengths[b] rows of each batch matter.  The host
rounds each batch up to whole 128-row chunks, bin-packs batches across the
8 cores to balance total chunk counts, and gathers each core's chunks into
one dense contiguous DRAM buffer.  On device, every chunk is streamed
through the PE as the moving operand of a skinny matmul whose stationary
operand is a [128, 8] weight tile: the single non-zero column holds
(row_valid ? 1/L : 0) for this chunk and selects which of the core's 8
output slots receives the partial sum (PSUM accumulates across all
chunks).  Because chunk routing lives entirely in the weight DATA, all 8
cores run one identical SPMD program regardless of the length pattern.
"""

import sys
import time

sys.path.insert(0, "/opt/trn_rl_repo")

import numpy as np

B, S, D = 32, 2048, 1024
N_CORES = 8
P = 128               # rows per chunk (= SBUF partitions)
SLOTS = 8             # output slots (max batches) per core
DMA_CHUNKS = 4        # chunks per dma_start (4 * 512KB = 2MB)

_compiled = {}        # C (chunks per core) -> Bacc program


def _build(C):
    """Build the SPMD Bass program for C chunks per core."""
    import concourse.bacc as bacc
    import concourse.mybir as mybir
    import concourse.tile as tile

    nc = bacc.Bacc("TRN2", debug=False, enable_asserts=False, num_devices=N_CORES)
    f32 = mybir.dt.float32
    feats = nc.dram_tensor("feats", [C * P, D], f32, kind="ExternalInput")
    w = nc.dram_tensor("w", [P, SLOTS * C], f32, kind="ExternalInput")
    out = nc.dram_tensor("out", [SLOTS, D], f32, kind="ExternalOutput")

    with tile.TileContext(nc) as tc:
        with (
            tc.tile_pool(name="wpool", bufs=1) as wpool,
            tc.tile_pool(name="fpool", bufs=3) as fpool,
            tc.tile_pool(name="opool", bufs=1) as opool,
            tc.tile_pool(name="psum", bufs=1, space="PSUM") as psum,
        ):
            wt = wpool.tile([P, SLOTS * C], f32)
            nc.sync.dma_start(wt[:], w.ap())

            acc0 = psum.tile([SLOTS, 512], f32)
            acc1 = psum.tile([SLOTS, 512], f32)

            chunk = 0
            for u0 in range(0, C, DMA_CHUNKS):
                nu = min(DMA_CHUNKS, C - u0)
                ft = fpool.tile([P, DMA_CHUNKS, D], f32, tag="ft")
                src = feats.ap()[u0 * P:(u0 + nu) * P, :].rearrange(
                    "(n p) d -> p n d", p=P
                )
                nc.sync.dma_start(ft[:, :nu, :], src)
                for j in range(nu):
                    lhsT = wt[:, chunk * SLOTS:(chunk + 1) * SLOTS]
                    nc.tensor.matmul(
                        acc0[:], lhsT, ft[:, j, 0:512],
                        start=(chunk == 0), stop=(chunk == C - 1),
                    )
                    nc.tensor.matmul(
                        acc1[:], lhsT, ft[:, j, 512:1024],
                        start=(chunk == 0), stop=(chunk == C - 1),
                    )
                    chunk += 1

            ot = opool.tile([SLOTS, D], f32)
            nc.vector.tensor_copy(ot[:, 0:512], acc0[:])
            nc.vector.tensor_copy(ot[:, 512:1024], acc1[:])
            nc.sync.dma_start(out.ap(), ot[:])

    nc.compile()
    return nc


def _plan(lengths):
    """Bin-pack batches onto cores; return (C, per-core batch/slot lists)."""
    nk = [(int(l) + P - 1) // P for l in lengths]
    order = sorted(range(B), key=lambda b: -nk[b])
    loads = [0] * N_CORES
    assigned = [[] for _ in range(N_CORES)]  # core -> list of batch ids
    for b in order:
        cands = [c for c in range(N_CORES) if len(assigned[c]) < SLOTS]
        c = min(cands, key=lambda c: loads[c])
        assigned[c].append(b)
        loads[c] += nk[b]
    C = max(loads)
    return C, assigned, nk


def _run(features, lengths, trace=False, trace_cores=None):
    from concourse.bass_utils import run_bass_kernel_spmd

    features = np.ascontiguousarray(np.asarray(features), dtype=np.float32)
    lengths = np.asarray(lengths).astype(np.int64)
    assert features.shape == (B, S, D), features.shape
    assert lengths.shape == (B,), lengths.shape

    C, assigned, nk = _plan(lengths)

    in_maps = []
    for c in range(N_CORES):
        fc = np.empty((C * P, D), np.float32)
        wc = np.zeros((P, SLOTS * C), np.float32)
        pos = 0  # chunk cursor within this core
        for j, b in enumerate(assigned[c]):
            L = int(lengths[b])
            rows = nk[b] * P
            fc[pos * P:pos * P + rows] = features[b, :rows]
            inv = np.float32(1.0) / np.float32(L)
            for k in range(nk[b]):
                valid = min(L - k * P, P)
                wc[0:valid, (pos + k) * SLOTS + j] = inv
            pos += nk[b]
        fc[pos * P:] = 0.0  # pad chunks (w columns already zero)
        in_maps.append({"feats": fc, "w": wc})

    if C not in _compiled:
        _compiled[C] = _build(C)
    nc = _compiled[C]

    res = run_bass_kernel_spmd(
        nc, in_maps, list(range(N_CORES)), trace=trace, trace_cores=trace_cores
    )

    pooled = np.empty((B, D), np.float32)
    for c in range(N_CORES):
        oc = res.results[c]["out"]
        for j, b in enumerate(assigned[c]):
            pooled[b] = oc[j]
    return pooled, res


def kernel(features, lengths):
    pooled, _ = _run(features, lengths)
    return (pooled, None)


if __name__ == "__main__":
    rng = np.random.default_rng(0)
    f = rng.standard_normal((B, S, D), dtype=np.float32)
    l = rng.integers(1, S + 1, size=(B,)).astype(np.int32)
    t0 = time.time()
    got, _ = _run(f, l)
    print(f"run took {time.time() - t0:.1f}s")
    mask = np.arange(S)[None, :] < l[:, None]
    exp = np.einsum("bsd,bs->bd", f, mask.astype(np.float64)) / l[:, None]
    rel = np.abs(got - exp) / np.maximum(np.abs(exp), 1e-6)
    print("max rel err:", rel.max())


# revision 23
# speedup vs baseline: 1.1891x; 1.1891x over previous
"""Masked mean-pool over ragged sequences on 8 Trainium2 NeuronCores.

features [32, 2048, 1024] f32, lengths [32] i32 -> pooled [32, 1024] f32
pooled[b] = mean(features[b, :lengths[b]], axis=0)

Only the first lengths[b] rows of each batch matter, so the host bin-packs
batches onto the 8 cores balancing exact row counts and packs each core's
valid rows DENSELY into one contiguous buffer (a 128-row chunk may span
several batches).  Traffic is ~sum(lengths)*4KB instead of the naive 256MB.

The fp32 rows are shipped as an exact fp16 (hi, lo) pair — same bytes, but
PE fp16 matmuls stream 4x faster than fp32 (which walrus emulates in two
half-rate passes).  Each chunk runs 4 matmuls (hi/lo x two 512-col halves)
against a [128, 8] one-hot fp16 mask tile whose column routes every row to
its batch's output slot; PSUM accumulates all chunks into [8, 512]x2, and
one per-partition 1/L scale on DVE finishes the mean.  Because routing
lives entirely in the mask DATA, all 8 cores run one identical SPMD
program for any length pattern.  All chunk DMAs are issued up-front
(whole stream resident in SBUF) in 512KB pieces alternating across the
two HWDGE rings, which measured fastest and smoothest at ~420GB/s/core.
"""

import sys
import time

sys.path.insert(0, "/opt/trn_rl_repo")

import numpy as np

B, S, D = 32, 2048, 1024
N_CORES = 8
P = 128               # rows per chunk (= SBUF partitions)
SLOTS = 8             # output slots (max batches) per core
DMA_CHUNKS = 1        # chunks per dma_start (512KB each)
NBLK = D // P         # feature-dim blocks in flip mode

MODE = "hl16"         # hl16 | mm_f32 | mm_f32r | flip_f32 | flip_f32r

_compiled = {}        # (C, mode, dma_chunks, bufs) -> Bacc program


def _build(C, mode, dma_chunks=DMA_CHUNKS, bufs=3):
    """Build the SPMD Bass program for C chunks per core."""
    import concourse.bacc as bacc
    import concourse.mybir as mybir
    import concourse.tile as tile

    flip = mode.startswith("flip")
    f32 = mybir.dt.float32
    f16 = mybir.dt.float16
    io_dt = mybir.dt.float32r if mode.endswith("f32r") else f32

    nc = bacc.Bacc("TRN2", debug=False, enable_asserts=False, num_devices=N_CORES)
    if mode == "hl16s":
        feats = nc.dram_tensor("feats", [2, C * P, D], f16, kind="ExternalInput")
        w = nc.dram_tensor("w", [P, SLOTS * C], f16, kind="ExternalInput")
        ilen = nc.dram_tensor("ilen", [SLOTS, 1], f32, kind="ExternalInput")
    elif mode == "hl16":
        feats = nc.dram_tensor("feats", [C * P, 2, D], f16, kind="ExternalInput")
        w = nc.dram_tensor("w", [P, SLOTS * C], f16, kind="ExternalInput")
        ilen = nc.dram_tensor("ilen", [SLOTS, 1], f32, kind="ExternalInput")
    else:
        feats = nc.dram_tensor("feats", [C * P, D], io_dt, kind="ExternalInput")
        w = nc.dram_tensor("w", [P, SLOTS * C], io_dt, kind="ExternalInput")
    out_shape = [D, SLOTS] if flip else [SLOTS, D]
    out = nc.dram_tensor("out", out_shape, f32, kind="ExternalOutput")

    with tile.TileContext(nc) as tc:
        with (
            tc.tile_pool(name="wpool", bufs=1) as wpool,
            tc.tile_pool(name="fpool", bufs=max(bufs, 1)) as fpool,
            tc.tile_pool(name="opool", bufs=1) as opool,
            tc.tile_pool(name="psum", bufs=1, space="PSUM") as psum,
        ):
            hl = mode in ("hl16", "hl16s")
            wt = wpool.tile([P, SLOTS * C], f16 if hl else io_dt)
            nc.scalar.dma_start(wt[:], w.ap())
            if hl:
                il = wpool.tile([SLOTS, 1], f32)
                nc.scalar.dma_start(il[:], ilen.ap())

            if flip:
                accs = [psum.tile([P, SLOTS], f32, tag=f"acc{b}", name=f"acc{b}")
                        for b in range(NBLK)]
            else:
                accs = [psum.tile([SLOTS, 512], f32, tag=f"acc{h}", name=f"acc{h}")
                        for h in range(2)]

            # ramped DMA group sizes: small at both ends — fast pipeline
            # fill at the start, short sem-wait->matmul->drain chain at the end
            head, tail = ([], []) if dma_chunks <= 2 else ([1, 1, 2], [2, 1, 1])
            groups = []
            left = C
            for g in head:
                if left - g < sum(tail):
                    break
                groups.append(g)
                left -= g
            tl = []
            for g in tail:
                if left <= 0:
                    break
                g = min(g, left)
                tl.append(g)
                left -= g
            while left > 0:
                g = min(dma_chunks, left)
                groups.append(g)
                left -= g
            groups += tl

            chunk = 0
            u0 = 0
            for gi, nu in enumerate(groups):
                eng = nc.sync if gi % 2 == 0 else nc.scalar
                # bufs=0: exact-fit slot per group (whole stream lives in
                # SBUF; every dma_start can issue up-front, no slot-gating)
                sz, tag = (nu, f"ft{gi}") if bufs == 0 else (dma_chunks, "ft")
                if mode == "hl16s":
                    fth = fpool.tile([P, sz, D], f16, tag="h" + tag, name=f"th{gi}")
                    ftl = fpool.tile([P, sz, D], f16, tag="l" + tag, name=f"tl{gi}")
                    for h, t in ((0, fth), (1, ftl)):
                        src = feats.ap()[h, u0 * P:(u0 + nu) * P, :].rearrange(
                            "(n p) d -> p n d", p=P
                        )
                        (nc.sync if h == 0 else nc.scalar).dma_start(
                            t[:, :nu, :], src)
                elif mode == "hl16":
                    ft = fpool.tile([P, sz, 2, D], f16, tag=tag, name=f"t{gi}")
                    src = feats.ap()[u0 * P:(u0 + nu) * P, :, :].rearrange(
                        "(n p) h d -> p n h d", p=P
                    )
                    eng.dma_start(ft[:, :nu, :, :], src)
                else:
                    ft = fpool.tile([P, sz, D], io_dt, tag=tag, name=f"t{gi}")
                    src = feats.ap()[u0 * P:(u0 + nu) * P, :].rearrange(
                        "(n p) d -> p n d", p=P
                    )
                    eng.dma_start(ft[:, :nu, :], src)
                for j in range(nu):
                    lw = wt[:, chunk * SLOTS:(chunk + 1) * SLOTS]
                    st, sp = chunk == 0, chunk == C - 1
                    if mode == "hl16s":
                        for h in range(2):
                            nc.tensor.matmul(accs[h][:], lw,
                                             fth[:, j, h * 512:(h + 1) * 512],
                                             start=st, stop=False)
                            nc.tensor.matmul(accs[h][:], lw,
                                             ftl[:, j, h * 512:(h + 1) * 512],
                                             start=False, stop=sp)
                    elif mode == "hl16":
                        for h in range(2):
                            nc.tensor.matmul(accs[h][:], lw,
                                             ft[:, j, 0, h * 512:(h + 1) * 512],
                                             start=st, stop=False)
                            nc.tensor.matmul(accs[h][:], lw,
                                             ft[:, j, 1, h * 512:(h + 1) * 512],
                                             start=False, stop=sp)
                    elif flip:
                        for b in range(NBLK):
                            nc.tensor.matmul(
                                accs[b][:], ft[:, j, b * P:(b + 1) * P], lw,
                                start=st, stop=sp,
                            )
                    else:
                        nc.tensor.matmul(accs[0][:], lw, ft[:, j, 0:512],
                                         start=st, stop=sp)
                        nc.tensor.matmul(accs[1][:], lw, ft[:, j, 512:1024],
                                         start=st, stop=sp)
                    chunk += 1
                u0 += nu

            if flip:
                ot = opool.tile([P, NBLK, SLOTS], f32)
                for b in range(NBLK):
                    nc.vector.tensor_copy(ot[:, b, :], accs[b][:])
                dst = out.ap().rearrange("(b p) s -> p b s", p=P)
                nc.sync.dma_start(dst, ot[:])
            else:
                ot = opool.tile([SLOTS, D], f32)
                if hl:
                    nc.vector.tensor_scalar_mul(ot[:, 0:512], accs[0][:], il[:])
                    nc.vector.tensor_scalar_mul(ot[:, 512:1024], accs[1][:], il[:])
                else:
                    nc.vector.tensor_copy(ot[:, 0:512], accs[0][:])
                    nc.vector.tensor_copy(ot[:, 512:1024], accs[1][:])
                nc.sync.dma_start(out.ap(), ot[:])

    nc.compile()
    return nc


def _plan(lengths):
    """Bin-pack batches onto cores balancing exact row counts.

    Rows of a core's batches are packed densely (a 128-row chunk may span
    several batches; the one-hot W column routes each row to its slot), so a
    core's chunk count is ceil(sum of its lengths / 128)."""
    order = sorted(range(B), key=lambda b: -int(lengths[b]))
    loads = [0] * N_CORES
    assigned = [[] for _ in range(N_CORES)]  # core -> list of batch ids
    for b in order:
        cands = [c for c in range(N_CORES) if len(assigned[c]) < SLOTS]
        c = min(cands, key=lambda c: loads[c])
        assigned[c].append(b)
        loads[c] += int(lengths[b])
    C = max((ld + P - 1) // P for ld in loads)
    return C, assigned


def _run(features, lengths, trace=False, trace_cores=None, mode=MODE,
         dma_chunks=DMA_CHUNKS, bufs=None):
    from concourse.bass_utils import run_bass_kernel_spmd

    features = np.ascontiguousarray(np.asarray(features), dtype=np.float32)
    lengths = np.asarray(lengths).astype(np.int64)
    assert features.shape == (B, S, D), features.shape
    assert lengths.shape == (B,), lengths.shape

    if mode in ("hl16", "hl16s"):
        amax = float(np.abs(features).max())
        if not (amax < 60000.0):  # fp16 split can't represent huge/inf values
            mode = "mm_f32"

    C, assigned = _plan(lengths)

    if bufs is None:
        # exact-fit (all chunks resident in SBUF, every DMA issued up-front)
        # when it fits; otherwise rotate a bounded number of slots
        bufs = 0 if C * 4 <= 168 else max(2, 168 // (4 * dma_chunks))

    hl = mode in ("hl16", "hl16s")
    in_maps = []
    for c in range(N_CORES):
        fc = np.zeros((C * P, D), np.float32)
        wc = np.zeros((P, C, SLOTS), np.float32)
        pos = 0  # row cursor within this core
        for j, b in enumerate(assigned[c]):
            L = int(lengths[b])
            Lc = min(L, S)  # reference sums at most S rows, divides by L
            fc[pos:pos + Lc] = features[b, :Lc]
            inv = np.float32(1.0) if hl else np.float32(1.0) / np.float32(L)
            rows = np.arange(pos, pos + Lc)
            wc[rows % P, rows // P, j] = inv
            pos += Lc
        wc = wc.reshape(P, C * SLOTS)
        if hl:
            if mode == "hl16s":
                fhl = np.empty((2, C * P, D), np.float16)
                hi = fc.astype(np.float16)
                fhl[0] = hi
                fhl[1] = (fc - hi.astype(np.float32)).astype(np.float16)
            else:
                fhl = np.empty((C * P, 2, D), np.float16)
                hi = fc.astype(np.float16)
                fhl[:, 0, :] = hi
                fhl[:, 1, :] = (fc - hi.astype(np.float32)).astype(np.float16)
            iv = np.ones((SLOTS, 1), np.float32)
            for j, b in enumerate(assigned[c]):
                iv[j, 0] = np.float32(1.0) / np.float32(int(lengths[b]))
            in_maps.append({"feats": fhl, "w": wc.astype(np.float16), "ilen": iv})
        else:
            in_maps.append({"feats": fc, "w": wc})

    key = (C, mode, dma_chunks, bufs)
    if key not in _compiled:
        _compiled[key] = _build(C, mode, dma_chunks, bufs)
    nc = _compiled[key]

    res = run_bass_kernel_spmd(
        nc, in_maps, list(range(N_CORES)), trace=trace, trace_cores=trace_cores
    )

    flip = mode.startswith("flip")
    pooled = np.empty((B, D), np.float32)
    for c in range(N_CORES):
        oc = res.results[c]["out"]
        for j, b in enumerate(assigned[c]):
            pooled[b] = oc[:, j] if flip else oc[j]
    return pooled, res


def kernel(features, lengths):
    pooled, _ = _run(features, lengths)
    return (pooled, None)


if __name__ == "__main__":
    rng = np.random.default_rng(0)
    f = rng.standard_normal((B, S, D), dtype=np.float32)
    l = rng.integers(1, S + 1, size=(B,)).astype(np.int32)
    t0 = time.time()
    got, _ = _run(f, l)
    print(f"run took {time.time() - t0:.1f}s")
    mask = np.arange(S)[None, :] < l[:, None]
    exp = np.einsum("bsd,bs->bd", f.astype(np.float64), mask.astype(np.float64)) / l[:, None]
    err = np.abs(got - exp)
    print("abs max err:", err.max(), " scale-rel:", err.max() / np.abs(exp).max())


# revision 24
# speedup vs baseline: 1.2247x; 1.0299x over previous
"""Masked mean-pool over ragged sequences on 8 Trainium2 NeuronCores.

features [32, 2048, 1024] f32, lengths [32] i32 -> pooled [32, 1024] f32
pooled[b] = mean(features[b, :lengths[b]], axis=0)

Only the first lengths[b] rows of each batch matter, so the host bin-packs
batches onto the 8 cores balancing exact row counts and packs each core's
valid rows DENSELY into one contiguous buffer (a 128-row chunk may span
several batches).  Traffic is ~sum(lengths)*4KB instead of the naive 256MB.

The fp32 rows are shipped as an exact fp16 (hi, lo) pair — same bytes, but
PE fp16 matmuls stream 4x faster than fp32 (which walrus emulates in two
half-rate passes).  Each chunk runs 4 matmuls (hi/lo x two 512-col halves)
against a [128, 8] one-hot fp16 mask tile whose column routes every row to
its batch's output slot; PSUM accumulates all chunks into [8, 512]x2, and
one per-partition 1/L scale on DVE finishes the mean.  Because routing
lives entirely in the mask DATA, all 8 cores run one identical SPMD
program for any length pattern.  All chunk DMAs are issued up-front
(whole stream resident in SBUF) in 512KB pieces alternating across the
two HWDGE rings, which measured fastest and smoothest at ~420GB/s/core.
"""

import sys
import time

sys.path.insert(0, "/opt/trn_rl_repo")

import numpy as np

B, S, D = 32, 2048, 1024
N_CORES = 8
P = 128               # rows per chunk (= SBUF partitions)
SLOTS = 8             # output slots (max batches) per core
DMA_CHUNKS = 1        # chunks per dma_start (512KB each)
NBLK = D // P         # feature-dim blocks in flip mode

MODE = "hl16"         # hl16 | mm_f32 | mm_f32r | flip_f32 | flip_f32r

_compiled = {}        # (C, mode, dma_chunks, bufs) -> Bacc program


def _build(C, mode, dma_chunks=DMA_CHUNKS, bufs=3):
    """Build the SPMD Bass program for C chunks per core."""
    import concourse.bacc as bacc
    import concourse.mybir as mybir
    import concourse.tile as tile

    flip = mode.startswith("flip")
    f32 = mybir.dt.float32
    f16 = mybir.dt.float16
    io_dt = mybir.dt.float32r if mode.endswith("f32r") else f32

    nc = bacc.Bacc("TRN2", debug=False, enable_asserts=False, num_devices=N_CORES)
    if mode == "hl16s":
        feats = nc.dram_tensor("feats", [2, C * P, D], f16, kind="ExternalInput")
        w = nc.dram_tensor("w", [P, SLOTS * C], f16, kind="ExternalInput")
        ilen = nc.dram_tensor("ilen", [SLOTS, 1], f32, kind="ExternalInput")
    elif mode == "hl16":
        feats = nc.dram_tensor("feats", [C * P, 2, D], f16, kind="ExternalInput")
        w = nc.dram_tensor("w", [P, SLOTS * C], f16, kind="ExternalInput")
        ilen = nc.dram_tensor("ilen", [SLOTS, 1], f32, kind="ExternalInput")
    else:
        feats = nc.dram_tensor("feats", [C * P, D], io_dt, kind="ExternalInput")
        w = nc.dram_tensor("w", [P, SLOTS * C], io_dt, kind="ExternalInput")
    out_shape = [D, SLOTS] if flip else [SLOTS, D]
    out = nc.dram_tensor("out", out_shape, f32, kind="ExternalOutput")

    with tile.TileContext(nc) as tc:
        with (
            tc.tile_pool(name="wpool", bufs=1) as wpool,
            tc.tile_pool(name="fpool", bufs=max(bufs, 1)) as fpool,
            tc.tile_pool(name="opool", bufs=1) as opool,
            tc.tile_pool(name="psum", bufs=1, space="PSUM") as psum,
        ):
            hl = mode in ("hl16", "hl16s")
            wt = wpool.tile([P, SLOTS * C], f16 if hl else io_dt)
            nc.scalar.dma_start(wt[:], w.ap())
            if hl:
                il = wpool.tile([SLOTS, 1], f32)
                nc.scalar.dma_start(il[:], ilen.ap())

            if flip:
                accs = [psum.tile([P, SLOTS], f32, tag=f"acc{b}", name=f"acc{b}")
                        for b in range(NBLK)]
            else:
                accs = [psum.tile([SLOTS, 512], f32, tag=f"acc{h}", name=f"acc{h}")
                        for h in range(2)]

            # ramped DMA group sizes: small at both ends — fast pipeline
            # fill at the start, short sem-wait->matmul->drain chain at the end
            head, tail = ([], []) if dma_chunks <= 2 else ([1, 1, 2], [2, 1, 1])
            groups = []
            left = C
            for g in head:
                if left - g < sum(tail):
                    break
                groups.append(g)
                left -= g
            tl = []
            for g in tail:
                if left <= 0:
                    break
                g = min(g, left)
                tl.append(g)
                left -= g
            while left > 0:
                g = min(dma_chunks, left)
                groups.append(g)
                left -= g
            groups += tl

            chunk = 0
            u0 = 0
            for gi, nu in enumerate(groups):
                eng = nc.sync if gi % 2 == 0 else nc.scalar
                # bufs=0: exact-fit slot per group (whole stream lives in
                # SBUF; every dma_start can issue up-front, no slot-gating)
                sz, tag = (nu, f"ft{gi}") if bufs == 0 else (dma_chunks, "ft")
                if mode == "hl16s":
                    fth = fpool.tile([P, sz, D], f16, tag="h" + tag, name=f"th{gi}")
                    ftl = fpool.tile([P, sz, D], f16, tag="l" + tag, name=f"tl{gi}")
                    for h, t in ((0, fth), (1, ftl)):
                        src = feats.ap()[h, u0 * P:(u0 + nu) * P, :].rearrange(
                            "(n p) d -> p n d", p=P
                        )
                        (nc.sync if h == 0 else nc.scalar).dma_start(
                            t[:, :nu, :], src)
                elif mode == "hl16":
                    ft = fpool.tile([P, sz, 2, D], f16, tag=tag, name=f"t{gi}")
                    src = feats.ap()[u0 * P:(u0 + nu) * P, :, :].rearrange(
                        "(n p) h d -> p n h d", p=P
                    )
                    eng.dma_start(ft[:, :nu, :, :], src)
                else:
                    ft = fpool.tile([P, sz, D], io_dt, tag=tag, name=f"t{gi}")
                    src = feats.ap()[u0 * P:(u0 + nu) * P, :].rearrange(
                        "(n p) d -> p n d", p=P
                    )
                    eng.dma_start(ft[:, :nu, :], src)
                for j in range(nu):
                    lw = wt[:, chunk * SLOTS:(chunk + 1) * SLOTS]
                    st, sp = chunk == 0, chunk == C - 1
                    if mode == "hl16s":
                        for h in range(2):
                            nc.tensor.matmul(accs[h][:], lw,
                                             fth[:, j, h * 512:(h + 1) * 512],
                                             start=st, stop=False)
                            nc.tensor.matmul(accs[h][:], lw,
                                             ftl[:, j, h * 512:(h + 1) * 512],
                                             start=False, stop=sp)
                    elif mode == "hl16":
                        for h in range(2):
                            nc.tensor.matmul(accs[h][:], lw,
                                             ft[:, j, 0, h * 512:(h + 1) * 512],
                                             start=st, stop=False)
                            nc.tensor.matmul(accs[h][:], lw,
                                             ft[:, j, 1, h * 512:(h + 1) * 512],
                                             start=False, stop=sp)
                    elif flip:
                        for b in range(NBLK):
                            nc.tensor.matmul(
                                accs[b][:], ft[:, j, b * P:(b + 1) * P], lw,
                                start=st, stop=sp,
                            )
                    else:
                        nc.tensor.matmul(accs[0][:], lw, ft[:, j, 0:512],
                                         start=st, stop=sp)
                        nc.tensor.matmul(accs[1][:], lw, ft[:, j, 512:1024],
                                         start=st, stop=sp)
                    chunk += 1
                u0 += nu

            if flip:
                ot = opool.tile([P, NBLK, SLOTS], f32)
                for b in range(NBLK):
                    nc.vector.tensor_copy(ot[:, b, :], accs[b][:])
                dst = out.ap().rearrange("(b p) s -> p b s", p=P)
                nc.sync.dma_start(dst, ot[:])
            else:
                ot = opool.tile([SLOTS, D], f32)
                if hl:
                    nc.vector.tensor_scalar_mul(ot[:, 0:512], accs[0][:], il[:])
                    nc.vector.tensor_scalar_mul(ot[:, 512:1024], accs[1][:], il[:])
                else:
                    nc.vector.tensor_copy(ot[:, 0:512], accs[0][:])
                    nc.vector.tensor_copy(ot[:, 512:1024], accs[1][:])
                nc.sync.dma_start(out.ap(), ot[:])

    nc.compile()
    return nc


def _plan(lengths):
    """Bin-pack batches onto cores balancing exact row counts.

    Rows of a core's batches are packed densely (a 128-row chunk may span
    several batches; the one-hot W column routes each row to its slot), so a
    core's chunk count is ceil(sum of its lengths / 128)."""
    order = sorted(range(B), key=lambda b: -int(lengths[b]))
    loads = [0] * N_CORES
    assigned = [[] for _ in range(N_CORES)]  # core -> list of batch ids
    for b in order:
        cands = [c for c in range(N_CORES) if len(assigned[c]) < SLOTS]
        c = min(cands, key=lambda c: loads[c])
        assigned[c].append(b)
        loads[c] += int(lengths[b])

    # local search: move/swap batches off the max-loaded core while it helps
    ideal = (sum(loads) + N_CORES * P - 1) // (N_CORES * P)
    for _ in range(200):
        hi = max(range(N_CORES), key=lambda c: loads[c])
        if (loads[hi] + P - 1) // P <= ideal:
            break
        best = None  # (new_pair_max, kind, ...)
        for lo in range(N_CORES):
            if lo == hi:
                continue
            for bi, b in enumerate(assigned[hi]):
                lb = int(lengths[b])
                if len(assigned[lo]) < SLOTS:
                    m = max(loads[hi] - lb, loads[lo] + lb)
                    if m < loads[hi] and (best is None or m < best[0]):
                        best = (m, "move", lo, bi, None)
                for bj, b2 in enumerate(assigned[lo]):
                    lb2 = int(lengths[b2])
                    m = max(loads[hi] - lb + lb2, loads[lo] + lb - lb2)
                    if m < loads[hi] and (best is None or m < best[0]):
                        best = (m, "swap", lo, bi, bj)
        if best is None:
            break
        _, kind, lo, bi, bj = best
        b = assigned[hi][bi]
        if kind == "move":
            assigned[hi].pop(bi)
            assigned[lo].append(b)
            loads[hi] -= int(lengths[b])
            loads[lo] += int(lengths[b])
        else:
            b2 = assigned[lo][bj]
            assigned[hi][bi], assigned[lo][bj] = b2, b
            d = int(lengths[b]) - int(lengths[b2])
            loads[hi] -= d
            loads[lo] += d

    C = max((ld + P - 1) // P for ld in loads)
    return C, assigned


def _run(features, lengths, trace=False, trace_cores=None, mode=MODE,
         dma_chunks=DMA_CHUNKS, bufs=None):
    from concourse.bass_utils import run_bass_kernel_spmd

    features = np.ascontiguousarray(np.asarray(features), dtype=np.float32)
    lengths = np.asarray(lengths).astype(np.int64)
    assert features.shape == (B, S, D), features.shape
    assert lengths.shape == (B,), lengths.shape

    if mode in ("hl16", "hl16s"):
        amax = float(np.abs(features).max())
        if not (amax < 60000.0):  # fp16 split can't represent huge/inf values
            mode = "mm_f32"

    C, assigned = _plan(lengths)

    if bufs is None:
        # exact-fit (all chunks resident in SBUF, every DMA issued up-front)
        # when it fits; otherwise rotate a bounded number of slots
        bufs = 0 if C * 4 <= 168 else max(2, 168 // (4 * dma_chunks))

    hl = mode in ("hl16", "hl16s")
    in_maps = []
    for c in range(N_CORES):
        fc = np.zeros((C * P, D), np.float32)
        wc = np.zeros((P, C, SLOTS), np.float32)
        pos = 0  # row cursor within this core
        for j, b in enumerate(assigned[c]):
            L = int(lengths[b])
            Lc = min(L, S)  # reference sums at most S rows, divides by L
            fc[pos:pos + Lc] = features[b, :Lc]
            inv = np.float32(1.0) if hl else np.float32(1.0) / np.float32(L)
            rows = np.arange(pos, pos + Lc)
            wc[rows % P, rows // P, j] = inv
            pos += Lc
        wc = wc.reshape(P, C * SLOTS)
        if hl:
            if mode == "hl16s":
                fhl = np.empty((2, C * P, D), np.float16)
                hi = fc.astype(np.float16)
                fhl[0] = hi
                fhl[1] = (fc - hi.astype(np.float32)).astype(np.float16)
            else:
                fhl = np.empty((C * P, 2, D), np.float16)
                hi = fc.astype(np.float16)
                fhl[:, 0, :] = hi
                fhl[:, 1, :] = (fc - hi.astype(np.float32)).astype(np.float16)
            iv = np.ones((SLOTS, 1), np.float32)
            for j, b in enumerate(assigned[c]):
                iv[j, 0] = np.float32(1.0) / np.float32(int(lengths[b]))
            in_maps.append({"feats": fhl, "w": wc.astype(np.float16), "ilen": iv})
        else:
            in_maps.append({"feats": fc, "w": wc})

    key = (C, mode, dma_chunks, bufs)
    if key not in _compiled:
        _compiled[key] = _build(C, mode, dma_chunks, bufs)
    nc = _compiled[key]

    res = run_bass_kernel_spmd(
        nc, in_maps, list(range(N_CORES)), trace=trace, trace_cores=trace_cores
    )

    flip = mode.startswith("flip")
    pooled = np.empty((B, D), np.float32)
    for c in range(N_CORES):
        oc = res.results[c]["out"]
        for j, b in enumerate(assigned[c]):
            pooled[b] = oc[:, j] if flip else oc[j]
    return pooled, res


def kernel(features, lengths):
    pooled, _ = _run(features, lengths)
    return (pooled, None)


if __name__ == "__main__":
    rng = np.random.default_rng(0)
    f = rng.standard_normal((B, S, D), dtype=np.float32)
    l = rng.integers(1, S + 1, size=(B,)).astype(np.int32)
    t0 = time.time()
    got, _ = _run(f, l)
    print(f"run took {time.time() - t0:.1f}s")
    mask = np.arange(S)[None, :] < l[:, None]
    exp = np.einsum("bsd,bs->bd", f.astype(np.float64), mask.astype(np.float64)) / l[:, None]
    err = np.abs(got - exp)
    print("abs max err:", err.max(), " scale-rel:", err.max() / np.abs(exp).max())


# revision 26
# speedup vs baseline: 1.2421x; 1.0142x over previous
"""Masked mean-pool over ragged sequences on 8 Trainium2 NeuronCores.

features [32, 2048, 1024] f32, lengths [32] i32 -> pooled [32, 1024] f32
pooled[b] = mean(features[b, :lengths[b]], axis=0)

Only the first lengths[b] rows of each batch matter, so the host bin-packs
batches onto the 8 cores balancing exact row counts and packs each core's
valid rows DENSELY into one contiguous buffer (a 128-row chunk may span
several batches).  Traffic is ~sum(lengths)*4KB instead of the naive 256MB.

The fp32 rows are shipped as an exact fp16 (hi, lo) pair — same bytes, but
PE fp16 matmuls stream 4x faster than fp32 (which walrus emulates in two
half-rate passes).  Each chunk runs 4 matmuls (hi/lo x two 512-col halves)
against a [128, 8] one-hot fp16 mask tile whose column routes every row to
its batch's output slot; PSUM accumulates all chunks into [8, 512]x2, and
one per-partition 1/L scale on DVE finishes the mean.  Because routing
lives entirely in the mask DATA, all 8 cores run one identical SPMD
program for any length pattern.  All chunk DMAs are issued up-front
(whole stream resident in SBUF) in 512KB pieces alternating across the
two HWDGE rings, which measured fastest and smoothest at ~420GB/s/core.
"""

import sys
import time

sys.path.insert(0, "/opt/trn_rl_repo")

import numpy as np

B, S, D = 32, 2048, 1024
N_CORES = 8
P = 128               # rows per chunk (= SBUF partitions)
SLOTS = 8             # output slots (max batches) per core
DMA_CHUNKS = 1        # chunks per dma_start (512KB each)
NBLK = D // P         # feature-dim blocks in flip mode

MODE = "hl16"         # hl16 | mm_f32 | mm_f32r | flip_f32 | flip_f32r

_compiled = {}        # (C, mode, dma_chunks, bufs) -> Bacc program


def _build(C, mode, dma_chunks=DMA_CHUNKS, bufs=3):
    """Build the SPMD Bass program for C chunks per core."""
    import concourse.bacc as bacc
    import concourse.mybir as mybir
    import concourse.tile as tile

    flip = mode.startswith("flip")
    base_mode = mode.split("@")[0]
    f32 = mybir.dt.float32
    f16 = mybir.dt.float16
    io_dt = mybir.dt.float32r if mode.endswith("f32r") else f32

    nc = bacc.Bacc("TRN2", debug=False, enable_asserts=False, num_devices=N_CORES)
    if base_mode == "hl16s":
        feats = nc.dram_tensor("feats", [2, C * P, D], f16, kind="ExternalInput")
        w = nc.dram_tensor("w", [P, SLOTS * C], f16, kind="ExternalInput")
        ilen = nc.dram_tensor("ilen", [SLOTS, 1], f32, kind="ExternalInput")
    elif base_mode == "hl16":
        feats = nc.dram_tensor("feats", [C * P, 2, D], f16, kind="ExternalInput")
        w = nc.dram_tensor("w", [P, SLOTS * C], f16, kind="ExternalInput")
        ilen = nc.dram_tensor("ilen", [SLOTS, 1], f32, kind="ExternalInput")
    else:
        feats = nc.dram_tensor("feats", [C * P, D], io_dt, kind="ExternalInput")
        w = nc.dram_tensor("w", [P, SLOTS * C], io_dt, kind="ExternalInput")
    out_shape = [D, SLOTS] if flip else [SLOTS, D]
    out = nc.dram_tensor("out", out_shape, f32, kind="ExternalOutput")

    with tile.TileContext(nc) as tc:
        with (
            tc.tile_pool(name="wpool", bufs=1) as wpool,
            tc.tile_pool(name="fpool", bufs=max(bufs, 1)) as fpool,
            tc.tile_pool(name="opool", bufs=1) as opool,
            tc.tile_pool(name="psum", bufs=1, space="PSUM") as psum,
        ):
            hl = base_mode in ("hl16", "hl16s")
            wt = wpool.tile([P, SLOTS * C], f16 if hl else io_dt)
            nc.scalar.dma_start(wt[:], w.ap())
            if hl:
                il = wpool.tile([SLOTS, 1], f32)
                nc.scalar.dma_start(il[:], ilen.ap())

            if flip:
                accs = [psum.tile([P, SLOTS], f32, tag=f"acc{b}", name=f"acc{b}")
                        for b in range(NBLK)]
            else:
                accs = [psum.tile([SLOTS, 512], f32, tag=f"acc{h}", name=f"acc{h}")
                        for h in range(2)]

            # ramped DMA group sizes: small at both ends — fast pipeline
            # fill at the start, short sem-wait->matmul->drain chain at the end
            head, tail = ([], []) if dma_chunks <= 2 else ([1, 1, 2], [2, 1, 1])
            groups = []
            left = C
            for g in head:
                if left - g < sum(tail):
                    break
                groups.append(g)
                left -= g
            tl = []
            for g in tail:
                if left <= 0:
                    break
                g = min(g, left)
                tl.append(g)
                left -= g
            while left > 0:
                g = min(dma_chunks, left)
                groups.append(g)
                left -= g
            groups += tl

            chunk = 0
            u0 = 0
            rings = {2: [nc.sync, nc.scalar],
                     3: [nc.sync, nc.scalar, nc.gpsimd],
                     1: [nc.sync]}[int(mode.split("@")[1]) if "@" in mode else 1]
            for gi, nu in enumerate(groups):
                eng = rings[gi % len(rings)]
                # bufs=0: exact-fit slot per group (whole stream lives in
                # SBUF; every dma_start can issue up-front, no slot-gating)
                sz, tag = (nu, f"ft{gi}") if bufs == 0 else (dma_chunks, "ft")
                if base_mode == "hl16s":
                    fth = fpool.tile([P, sz, D], f16, tag="h" + tag, name=f"th{gi}")
                    ftl = fpool.tile([P, sz, D], f16, tag="l" + tag, name=f"tl{gi}")
                    for h, t in ((0, fth), (1, ftl)):
                        src = feats.ap()[h, u0 * P:(u0 + nu) * P, :].rearrange(
                            "(n p) d -> p n d", p=P
                        )
                        (nc.sync if h == 0 else nc.scalar).dma_start(
                            t[:, :nu, :], src)
                elif base_mode == "hl16":
                    ft = fpool.tile([P, sz, 2, D], f16, tag=tag, name=f"t{gi}")
                    src = feats.ap()[u0 * P:(u0 + nu) * P, :, :].rearrange(
                        "(n p) h d -> p n h d", p=P
                    )
                    eng.dma_start(ft[:, :nu, :, :], src)
                else:
                    ft = fpool.tile([P, sz, D], io_dt, tag=tag, name=f"t{gi}")
                    src = feats.ap()[u0 * P:(u0 + nu) * P, :].rearrange(
                        "(n p) d -> p n d", p=P
                    )
                    eng.dma_start(ft[:, :nu, :], src)
                for j in range(nu):
                    lw = wt[:, chunk * SLOTS:(chunk + 1) * SLOTS]
                    st, sp = chunk == 0, chunk == C - 1
                    if base_mode == "hl16s":
                        for h in range(2):
                            nc.tensor.matmul(accs[h][:], lw,
                                             fth[:, j, h * 512:(h + 1) * 512],
                                             start=st, stop=False)
                            nc.tensor.matmul(accs[h][:], lw,
                                             ftl[:, j, h * 512:(h + 1) * 512],
                                             start=False, stop=sp)
                    elif base_mode == "hl16":
                        for h in range(2):
                            nc.tensor.matmul(accs[h][:], lw,
                                             ft[:, j, 0, h * 512:(h + 1) * 512],
                                             start=st, stop=False)
                            nc.tensor.matmul(accs[h][:], lw,
                                             ft[:, j, 1, h * 512:(h + 1) * 512],
                                             start=False, stop=sp)
                    elif flip:
                        for b in range(NBLK):
                            nc.tensor.matmul(
                                accs[b][:], ft[:, j, b * P:(b + 1) * P], lw,
                                start=st, stop=sp,
                            )
                    else:
                        nc.tensor.matmul(accs[0][:], lw, ft[:, j, 0:512],
                                         start=st, stop=sp)
                        nc.tensor.matmul(accs[1][:], lw, ft[:, j, 512:1024],
                                         start=st, stop=sp)
                    chunk += 1
                u0 += nu

            if flip:
                ot = opool.tile([P, NBLK, SLOTS], f32)
                for b in range(NBLK):
                    nc.vector.tensor_copy(ot[:, b, :], accs[b][:])
                dst = out.ap().rearrange("(b p) s -> p b s", p=P)
                nc.sync.dma_start(dst, ot[:])
            else:
                ot = opool.tile([SLOTS, D], f32)
                if hl:
                    nc.vector.tensor_scalar_mul(ot[:, 0:512], accs[0][:], il[:])
                    nc.vector.tensor_scalar_mul(ot[:, 512:1024], accs[1][:], il[:])
                else:
                    nc.vector.tensor_copy(ot[:, 0:512], accs[0][:])
                    nc.vector.tensor_copy(ot[:, 512:1024], accs[1][:])
                nc.sync.dma_start(out.ap(), ot[:])

    nc.compile()
    return nc


def _plan(lengths):
    """Bin-pack batches onto cores balancing exact row counts.

    Rows of a core's batches are packed densely (a 128-row chunk may span
    several batches; the one-hot W column routes each row to its slot), so a
    core's chunk count is ceil(sum of its lengths / 128)."""
    order = sorted(range(B), key=lambda b: -int(lengths[b]))
    loads = [0] * N_CORES
    assigned = [[] for _ in range(N_CORES)]  # core -> list of batch ids
    for b in order:
        cands = [c for c in range(N_CORES) if len(assigned[c]) < SLOTS]
        c = min(cands, key=lambda c: loads[c])
        assigned[c].append(b)
        loads[c] += int(lengths[b])

    # local search: move/swap batches off the max-loaded core while it helps
    ideal = (sum(loads) + N_CORES * P - 1) // (N_CORES * P)
    for _ in range(200):
        hi = max(range(N_CORES), key=lambda c: loads[c])
        if (loads[hi] + P - 1) // P <= ideal:
            break
        best = None  # (new_pair_max, kind, ...)
        for lo in range(N_CORES):
            if lo == hi:
                continue
            for bi, b in enumerate(assigned[hi]):
                lb = int(lengths[b])
                if len(assigned[lo]) < SLOTS:
                    m = max(loads[hi] - lb, loads[lo] + lb)
                    if m < loads[hi] and (best is None or m < best[0]):
                        best = (m, "move", lo, bi, None)
                for bj, b2 in enumerate(assigned[lo]):
                    lb2 = int(lengths[b2])
                    m = max(loads[hi] - lb + lb2, loads[lo] + lb - lb2)
                    if m < loads[hi] and (best is None or m < best[0]):
                        best = (m, "swap", lo, bi, bj)
        if best is None:
            break
        _, kind, lo, bi, bj = best
        b = assigned[hi][bi]
        if kind == "move":
            assigned[hi].pop(bi)
            assigned[lo].append(b)
            loads[hi] -= int(lengths[b])
            loads[lo] += int(lengths[b])
        else:
            b2 = assigned[lo][bj]
            assigned[hi][bi], assigned[lo][bj] = b2, b
            d = int(lengths[b]) - int(lengths[b2])
            loads[hi] -= d
            loads[lo] += d

    C = max((ld + P - 1) // P for ld in loads)
    return C, assigned


def _run(features, lengths, trace=False, trace_cores=None, mode=MODE,
         dma_chunks=DMA_CHUNKS, bufs=None):
    from concourse.bass_utils import run_bass_kernel_spmd

    features = np.ascontiguousarray(np.asarray(features), dtype=np.float32)
    lengths = np.asarray(lengths).astype(np.int64)
    assert features.shape == (B, S, D), features.shape
    assert lengths.shape == (B,), lengths.shape

    if mode.split("@")[0] in ("hl16", "hl16s"):
        amax = float(np.abs(features).max())
        if not (amax < 60000.0):  # fp16 split can't represent huge/inf values
            mode = "mm_f32"

    C, assigned = _plan(lengths)

    if bufs is None:
        # exact-fit (all chunks resident in SBUF, every DMA issued up-front)
        # when it fits; otherwise rotate a bounded number of slots
        bufs = 0 if C * 4 <= 168 else max(2, 168 // (4 * dma_chunks))

    hl = mode.split("@")[0] in ("hl16", "hl16s")
    in_maps = []
    for c in range(N_CORES):
        fc = np.zeros((C * P, D), np.float32)
        wc = np.zeros((P, C, SLOTS), np.float32)
        pos = 0  # row cursor within this core
        for j, b in enumerate(assigned[c]):
            L = int(lengths[b])
            Lc = min(L, S)  # reference sums at most S rows, divides by L
            fc[pos:pos + Lc] = features[b, :Lc]
            inv = np.float32(1.0) if hl else np.float32(1.0) / np.float32(L)
            rows = np.arange(pos, pos + Lc)
            wc[rows % P, rows // P, j] = inv
            pos += Lc
        wc = wc.reshape(P, C * SLOTS)
        if hl:
            if mode.split("@")[0] == "hl16s":
                fhl = np.empty((2, C * P, D), np.float16)
                hi = fc.astype(np.float16)
                fhl[0] = hi
                fhl[1] = (fc - hi.astype(np.float32)).astype(np.float16)
            else:
                fhl = np.empty((C * P, 2, D), np.float16)
                hi = fc.astype(np.float16)
                fhl[:, 0, :] = hi
                fhl[:, 1, :] = (fc - hi.astype(np.float32)).astype(np.float16)
            iv = np.ones((SLOTS, 1), np.float32)
            for j, b in enumerate(assigned[c]):
                iv[j, 0] = np.float32(1.0) / np.float32(int(lengths[b]))
            in_maps.append({"feats": fhl, "w": wc.astype(np.float16), "ilen": iv})
        else:
            in_maps.append({"feats": fc, "w": wc})

    key = (C, mode, dma_chunks, bufs)
    if key not in _compiled:
        _compiled[key] = _build(C, mode, dma_chunks, bufs)
    nc = _compiled[key]

    res = run_bass_kernel_spmd(
        nc, in_maps, list(range(N_CORES)), trace=trace, trace_cores=trace_cores
    )

    flip = mode.startswith("flip")
    pooled = np.empty((B, D), np.float32)
    for c in range(N_CORES):
        oc = res.results[c]["out"]
        for j, b in enumerate(assigned[c]):
            pooled[b] = oc[:, j] if flip else oc[j]
    return pooled, res


def kernel(features, lengths):
    pooled, _ = _run(features, lengths)
    return (pooled, None)


if __name__ == "__main__":
    rng = np.random.default_rng(0)
    f = rng.standard_normal((B, S, D), dtype=np.float32)
    l = rng.integers(1, S + 1, size=(B,)).astype(np.int32)
    t0 = time.time()
    got, _ = _run(f, l)
    print(f"run took {time.time() - t0:.1f}s")
    mask = np.arange(S)[None, :] < l[:, None]
    exp = np.einsum("bsd,bs->bd", f.astype(np.float64), mask.astype(np.float64)) / l[:, None]
    err = np.abs(got - exp)
    print("abs max err:", err.max(), " scale-rel:", err.max() / np.abs(exp).max())


# revision 27
# speedup vs baseline: 1.2470x; 1.0039x over previous
"""Masked mean-pool over ragged sequences on 8 Trainium2 NeuronCores.

features [32, 2048, 1024] f32, lengths [32] i32 -> pooled [32, 1024] f32
pooled[b] = mean(features[b, :lengths[b]], axis=0)

Only the first lengths[b] rows of each batch matter, so the host bin-packs
batches onto the 8 cores balancing exact row counts and packs each core's
valid rows DENSELY into one contiguous buffer (a 128-row chunk may span
several batches).  Traffic is ~sum(lengths)*4KB instead of the naive 256MB.

The fp32 rows are shipped as an exact fp16 (hi, lo) pair — same bytes, but
PE fp16 matmuls stream 4x faster than fp32 (which walrus emulates in two
half-rate passes).  Each chunk runs 4 matmuls (hi/lo x two 512-col halves)
against a [128, 8] one-hot fp16 mask tile whose column routes every row to
its batch's output slot; PSUM accumulates all chunks into [8, 512]x2, and
one per-partition 1/L scale on DVE finishes the mean.  Because routing
lives entirely in the mask DATA, all 8 cores run one identical SPMD
program for any length pattern.  All chunk DMAs are issued up-front
(whole stream resident in SBUF) in 512KB pieces alternating across the
two HWDGE rings, which measured fastest and smoothest at ~420GB/s/core.
"""

import sys
import time

sys.path.insert(0, "/opt/trn_rl_repo")

import numpy as np

B, S, D = 32, 2048, 1024
N_CORES = 8
P = 128               # rows per chunk (= SBUF partitions)
SLOTS = 8             # output slots (max batches) per core
DMA_CHUNKS = 1        # chunks per dma_start (512KB each)
NBLK = D // P         # feature-dim blocks in flip mode

MODE = "hl16"         # hl16 | mm_f32 | mm_f32r | flip_f32 | flip_f32r

_compiled = {}        # (C, mode, dma_chunks, bufs) -> Bacc program


def _build(C, mode, dma_chunks=DMA_CHUNKS, bufs=3):
    """Build the SPMD Bass program for C chunks per core."""
    import concourse.bacc as bacc
    import concourse.mybir as mybir
    import concourse.tile as tile

    flip = mode.startswith("flip")
    base_mode = mode.split("@")[0]
    f32 = mybir.dt.float32
    f16 = mybir.dt.float16
    io_dt = mybir.dt.float32r if mode.endswith("f32r") else f32

    nc = bacc.Bacc("TRN2", debug=False, enable_asserts=False, num_devices=N_CORES)
    if base_mode == "hl16s":
        feats = nc.dram_tensor("feats", [2, C * P, D], f16, kind="ExternalInput")
        w = nc.dram_tensor("w", [P, SLOTS * C], f16, kind="ExternalInput")
        ilen = nc.dram_tensor("ilen", [SLOTS, 1], f32, kind="ExternalInput")
    elif base_mode == "hl16":
        feats = nc.dram_tensor("feats", [C * P, 2, D], f16, kind="ExternalInput")
        w = nc.dram_tensor("w", [P, SLOTS * C], f16, kind="ExternalInput")
        ilen = nc.dram_tensor("ilen", [SLOTS, 1], f32, kind="ExternalInput")
    else:
        feats = nc.dram_tensor("feats", [C * P, D], io_dt, kind="ExternalInput")
        w = nc.dram_tensor("w", [P, SLOTS * C], io_dt, kind="ExternalInput")
    out_shape = [D, SLOTS] if flip else [SLOTS, D]
    out = nc.dram_tensor("out", out_shape, f32, kind="ExternalOutput")

    with tile.TileContext(nc) as tc:
        with (
            tc.tile_pool(name="wpool", bufs=1) as wpool,
            tc.tile_pool(name="fpool", bufs=max(bufs, 1)) as fpool,
            tc.tile_pool(name="opool", bufs=1) as opool,
            tc.tile_pool(name="psum", bufs=1, space="PSUM") as psum,
        ):
            hl = base_mode in ("hl16", "hl16s")
            wt = wpool.tile([P, SLOTS * C], f16 if hl else io_dt)
            nc.scalar.dma_start(wt[:], w.ap())
            if hl:
                il = wpool.tile([SLOTS, 1], f32)
                nc.scalar.dma_start(il[:], ilen.ap())

            if flip:
                accs = [psum.tile([P, SLOTS], f32, tag=f"acc{b}", name=f"acc{b}")
                        for b in range(NBLK)]
            else:
                accs = [psum.tile([SLOTS, 512], f32, tag=f"acc{h}", name=f"acc{h}")
                        for h in range(2)]

            # ramped DMA group sizes: small at both ends — fast pipeline
            # fill at the start, short sem-wait->matmul->drain chain at the end
            head, tail = ([], []) if dma_chunks <= 2 else ([1, 1, 2], [2, 1, 1])
            groups = []
            left = C
            for g in head:
                if left - g < sum(tail):
                    break
                groups.append(g)
                left -= g
            tl = []
            for g in tail:
                if left <= 0:
                    break
                g = min(g, left)
                tl.append(g)
                left -= g
            while left > 0:
                g = min(dma_chunks, left)
                groups.append(g)
                left -= g
            groups += tl

            chunk = 0
            u0 = 0
            rings = {2: [nc.sync, nc.scalar],
                     3: [nc.sync, nc.scalar, nc.gpsimd],
                     1: [nc.sync]}[int(mode.split("@")[1]) if "@" in mode else 1]
            for gi, nu in enumerate(groups):
                eng = rings[gi % len(rings)]
                # bufs=0: exact-fit slot per group (whole stream lives in
                # SBUF; every dma_start can issue up-front, no slot-gating)
                sz, tag = (nu, f"ft{gi}") if bufs == 0 else (dma_chunks, "ft")
                if base_mode == "hl16s":
                    fth = fpool.tile([P, sz, D], f16, tag="h" + tag, name=f"th{gi}")
                    ftl = fpool.tile([P, sz, D], f16, tag="l" + tag, name=f"tl{gi}")
                    for h, t in ((0, fth), (1, ftl)):
                        src = feats.ap()[h, u0 * P:(u0 + nu) * P, :].rearrange(
                            "(n p) d -> p n d", p=P
                        )
                        (nc.sync if h == 0 else nc.scalar).dma_start(
                            t[:, :nu, :], src)
                elif base_mode == "hl16":
                    ft = fpool.tile([P, sz, 2, D], f16, tag=tag, name=f"t{gi}")
                    src = feats.ap()[u0 * P:(u0 + nu) * P, :, :].rearrange(
                        "(n p) h d -> p n h d", p=P
                    )
                    eng.dma_start(ft[:, :nu, :, :], src)
                else:
                    ft = fpool.tile([P, sz, D], io_dt, tag=tag, name=f"t{gi}")
                    src = feats.ap()[u0 * P:(u0 + nu) * P, :].rearrange(
                        "(n p) d -> p n d", p=P
                    )
                    eng.dma_start(ft[:, :nu, :], src)
                for j in range(nu):
                    lw = wt[:, chunk * SLOTS:(chunk + 1) * SLOTS]
                    st, sp = chunk == 0, chunk == C - 1
                    if base_mode == "hl16s":
                        for h in range(2):
                            nc.tensor.matmul(accs[h][:], lw,
                                             fth[:, j, h * 512:(h + 1) * 512],
                                             start=st, stop=False)
                            nc.tensor.matmul(accs[h][:], lw,
                                             ftl[:, j, h * 512:(h + 1) * 512],
                                             start=False, stop=sp)
                    elif base_mode == "hl16":
                        for h in range(2):
                            nc.tensor.matmul(accs[h][:], lw,
                                             ft[:, j, 0, h * 512:(h + 1) * 512],
                                             start=st, stop=False)
                            nc.tensor.matmul(accs[h][:], lw,
                                             ft[:, j, 1, h * 512:(h + 1) * 512],
                                             start=False, stop=sp)
                    elif flip:
                        for b in range(NBLK):
                            nc.tensor.matmul(
                                accs[b][:], ft[:, j, b * P:(b + 1) * P], lw,
                                start=st, stop=sp,
                            )
                    else:
                        nc.tensor.matmul(accs[0][:], lw, ft[:, j, 0:512],
                                         start=st, stop=sp)
                        nc.tensor.matmul(accs[1][:], lw, ft[:, j, 512:1024],
                                         start=st, stop=sp)
                    chunk += 1
                u0 += nu

            if flip:
                ot = opool.tile([P, NBLK, SLOTS], f32)
                for b in range(NBLK):
                    nc.vector.tensor_copy(ot[:, b, :], accs[b][:])
                dst = out.ap().rearrange("(b p) s -> p b s", p=P)
                nc.sync.dma_start(dst, ot[:])
            else:
                # per-half epilogue: half 0's scale+store overlaps half 1's
                # final matmuls (separate PSUM banks and HWDGE rings)
                ot = opool.tile([SLOTS, D], f32)
                for h, oeng in ((0, nc.sync), (1, nc.scalar)):
                    dst = ot[:, h * 512:(h + 1) * 512]
                    if hl:
                        nc.vector.tensor_scalar_mul(dst, accs[h][:], il[:])
                    else:
                        nc.vector.tensor_copy(dst, accs[h][:])
                    oeng.dma_start(out.ap()[:, h * 512:(h + 1) * 512], dst)

    nc.compile()
    return nc


def _plan(lengths):
    """Bin-pack batches onto cores balancing exact row counts.

    Rows of a core's batches are packed densely (a 128-row chunk may span
    several batches; the one-hot W column routes each row to its slot), so a
    core's chunk count is ceil(sum of its lengths / 128)."""
    order = sorted(range(B), key=lambda b: -int(lengths[b]))
    loads = [0] * N_CORES
    assigned = [[] for _ in range(N_CORES)]  # core -> list of batch ids
    for b in order:
        cands = [c for c in range(N_CORES) if len(assigned[c]) < SLOTS]
        c = min(cands, key=lambda c: loads[c])
        assigned[c].append(b)
        loads[c] += int(lengths[b])

    # local search: move/swap batches off the max-loaded core while it helps
    ideal = (sum(loads) + N_CORES * P - 1) // (N_CORES * P)
    for _ in range(200):
        hi = max(range(N_CORES), key=lambda c: loads[c])
        if (loads[hi] + P - 1) // P <= ideal:
            break
        best = None  # (new_pair_max, kind, ...)
        for lo in range(N_CORES):
            if lo == hi:
                continue
            for bi, b in enumerate(assigned[hi]):
                lb = int(lengths[b])
                if len(assigned[lo]) < SLOTS:
                    m = max(loads[hi] - lb, loads[lo] + lb)
                    if m < loads[hi] and (best is None or m < best[0]):
                        best = (m, "move", lo, bi, None)
                for bj, b2 in enumerate(assigned[lo]):
                    lb2 = int(lengths[b2])
                    m = max(loads[hi] - lb + lb2, loads[lo] + lb - lb2)
                    if m < loads[hi] and (best is None or m < best[0]):
                        best = (m, "swap", lo, bi, bj)
        if best is None:
            break
        _, kind, lo, bi, bj = best
        b = assigned[hi][bi]
        if kind == "move":
            assigned[hi].pop(bi)
            assigned[lo].append(b)
            loads[hi] -= int(lengths[b])
            loads[lo] += int(lengths[b])
        else:
            b2 = assigned[lo][bj]
            assigned[hi][bi], assigned[lo][bj] = b2, b
            d = int(lengths[b]) - int(lengths[b2])
            loads[hi] -= d
            loads[lo] += d

    C = max((ld + P - 1) // P for ld in loads)
    return C, assigned


def _run(features, lengths, trace=False, trace_cores=None, mode=MODE,
         dma_chunks=DMA_CHUNKS, bufs=None):
    from concourse.bass_utils import run_bass_kernel_spmd

    features = np.ascontiguousarray(np.asarray(features), dtype=np.float32)
    lengths = np.asarray(lengths).astype(np.int64)
    assert features.shape == (B, S, D), features.shape
    assert lengths.shape == (B,), lengths.shape

    if mode.split("@")[0] in ("hl16", "hl16s"):
        amax = float(np.abs(features).max())
        if not (amax < 60000.0):  # fp16 split can't represent huge/inf values
            mode = "mm_f32"

    C, assigned = _plan(lengths)

    if bufs is None:
        # exact-fit (all chunks resident in SBUF, every DMA issued up-front)
        # when it fits; otherwise rotate a bounded number of slots
        bufs = 0 if C * 4 <= 168 else max(2, 168 // (4 * dma_chunks))

    hl = mode.split("@")[0] in ("hl16", "hl16s")
    in_maps = []
    for c in range(N_CORES):
        fc = np.zeros((C * P, D), np.float32)
        wc = np.zeros((P, C, SLOTS), np.float32)
        pos = 0  # row cursor within this core
        for j, b in enumerate(assigned[c]):
            L = int(lengths[b])
            Lc = min(L, S)  # reference sums at most S rows, divides by L
            fc[pos:pos + Lc] = features[b, :Lc]
            inv = np.float32(1.0) if hl else np.float32(1.0) / np.float32(L)
            rows = np.arange(pos, pos + Lc)
            wc[rows % P, rows // P, j] = inv
            pos += Lc
        wc = wc.reshape(P, C * SLOTS)
        if hl:
            if mode.split("@")[0] == "hl16s":
                fhl = np.empty((2, C * P, D), np.float16)
                hi = fc.astype(np.float16)
                fhl[0] = hi
                fhl[1] = (fc - hi.astype(np.float32)).astype(np.float16)
            else:
                fhl = np.empty((C * P, 2, D), np.float16)
                hi = fc.astype(np.float16)
                fhl[:, 0, :] = hi
                fhl[:, 1, :] = (fc - hi.astype(np.float32)).astype(np.float16)
            iv = np.ones((SLOTS, 1), np.float32)
            for j, b in enumerate(assigned[c]):
                iv[j, 0] = np.float32(1.0) / np.float32(int(lengths[b]))
            in_maps.append({"feats": fhl, "w": wc.astype(np.float16), "ilen": iv})
        else:
            in_maps.append({"feats": fc, "w": wc})

    key = (C, mode, dma_chunks, bufs)
    if key not in _compiled:
        _compiled[key] = _build(C, mode, dma_chunks, bufs)
    nc = _compiled[key]

    res = run_bass_kernel_spmd(
        nc, in_maps, list(range(N_CORES)), trace=trace, trace_cores=trace_cores
    )

    flip = mode.startswith("flip")
    pooled = np.empty((B, D), np.float32)
    for c in range(N_CORES):
        oc = res.results[c]["out"]
        for j, b in enumerate(assigned[c]):
            pooled[b] = oc[:, j] if flip else oc[j]
    return pooled, res


def kernel(features, lengths):
    pooled, _ = _run(features, lengths)
    return (pooled, None)


if __name__ == "__main__":
    rng = np.random.default_rng(0)
    f = rng.standard_normal((B, S, D), dtype=np.float32)
    l = rng.integers(1, S + 1, size=(B,)).astype(np.int32)
    t0 = time.time()
    got, _ = _run(f, l)
    print(f"run took {time.time() - t0:.1f}s")
    mask = np.arange(S)[None, :] < l[:, None]
    exp = np.einsum("bsd,bs->bd", f.astype(np.float64), mask.astype(np.float64)) / l[:, None]
    err = np.abs(got - exp)
    print("abs max err:", err.max(), " scale-rel:", err.max() / np.abs(exp).max())
